# revision 6
# baseline (speedup 1.0000x reference)
# Causal self-attention on 8 NeuronCores (Trainium2, Bass/Tile).
#
# Sharding: core c -> batch b = c//4, head-group hg = c%4 (4 of 16 heads).
# Each core computes Q/K/V projections for its heads, causal attention, and
# a partial output projection (its heads' rows of W_out). Host sums the 4
# partials per batch (Megatron-style TP combine) and adds nothing else
# (b_out is folded in via the `bo` input on hg==0 cores).
#
# Device layouts (host-prepped, bf16 matmul operands):
#   xT  [1024, 2048]  x[b].T            -> rhs/lhsT chunks over e
#   wq/wk/wv [1024, 256]  W_qkv col-slices (wq pre-scaled by 1/sqrt(D))
#   wo  [256, 1024]   W_out row-slice
# Attention runs in S^T = K Q^T layout ([k, q]); the PV matmul uses
# lhsT = [V | 1] so PSUM row 64 accumulates the softmax denominator.

import os
import numpy as np
import ml_dtypes

import concourse.bass as bass
import concourse.mybir as mybir
import concourse.tile as tile
from concourse import bacc
from concourse.bass_utils import run_bass_kernel_spmd

F32 = mybir.dt.float32
BF16 = mybir.dt.bfloat16
AF = mybir.ActivationFunctionType
OP = mybir.AluOpType

T = 2048
E = 1024
D = 64
NH = 16
H_CORE = 4          # heads per core
PAIRS = 2           # head pairs per core
EC = E // 128       # 8 e-chunks
NT4 = T // 512      # 4 t-tiles of 512
NKT = T // 128      # 16 k-tiles of 128

_cache = {}


def _build():
    nc = bacc.Bacc(None, target_bir_lowering=False)
    xT = nc.declare_dram_parameter("xT", [E, T], BF16, isOutput=False)
    wq = nc.declare_dram_parameter("wq", [E, 256], BF16, isOutput=False)
    wk = nc.declare_dram_parameter("wk", [E, 256], BF16, isOutput=False)
    wv = nc.declare_dram_parameter("wv", [E, 256], BF16, isOutput=False)
    wo = nc.declare_dram_parameter("wo", [256, E], BF16, isOutput=False)
    bq = nc.declare_dram_parameter("bq", [128, 2], F32, isOutput=False)
    bk = nc.declare_dram_parameter("bk", [128, 2], F32, isOutput=False)
    bv = nc.declare_dram_parameter("bv", [1, 256], F32, isOutput=False)
    bo = nc.declare_dram_parameter("bo", [1, E], F32, isOutput=False)
    out = nc.declare_dram_parameter("out", [T, E], F32, isOutput=True)

    xT_r = xT.rearrange("(c p) t -> p c t", p=128)
    wq_r = wq.rearrange("(c p) m -> p c m", p=128)
    wk_r = wk.rearrange("(c p) m -> p c m", p=128)
    wv_r = wv.rearrange("(c p) m -> p c m", p=128)
    wo_r = wo.rearrange("(c p) n -> p c n", p=128)

    with tile.TileContext(nc) as tc:
        with (
            tc.tile_pool(name="w", bufs=1) as w,
            tc.tile_pool(name="pt", bufs=4) as ptp,
            tc.tile_pool(name="misc", bufs=2) as misc,
            tc.tile_pool(name="ob", bufs=3) as obp,
        ):
            # ---- static tiles ----
            XT = w.tile([128, EC, T], BF16)
            WQ = w.tile([128, EC, 256], BF16)
            WK = w.tile([128, EC, 256], BF16)
            WV = w.tile([128, EC, 256], BF16)
            WO = w.tile([128, 2, E], BF16)
            BQ = w.tile([128, 2], F32)
            BK = w.tile([128, 2], F32)
            BV1 = w.tile([1, 256], F32)
            BO1 = w.tile([1, E], F32)
            for c in range(EC):
                nc.sync.dma_start(XT[:, c, :], xT_r[:, c, :])
            nc.sync.dma_start(WQ[:], wq_r[:])
            nc.sync.dma_start(WK[:], wk_r[:])
            nc.sync.dma_start(WV[:], wv_r[:])
            nc.sync.dma_start(WO[:], wo_r[:])
            nc.sync.dma_start(BQ[:], bq[:])
            nc.sync.dma_start(BK[:], bk[:])
            nc.sync.dma_start(BV1[:], bv[:])
            nc.sync.dma_start(BO1[:], bo[:])
            BVB = w.tile([128, 256], F32)
            nc.gpsimd.partition_broadcast(BVB[:], BV1[:])
            BOB = w.tile([128, E], F32)
            nc.gpsimd.partition_broadcast(BOB[:], BO1[:])

            # triangular keep-mask [k, j]: keep j >= k
            M01 = w.tile([128, 128], BF16)
            nc.vector.memset(M01[:], 1.0)
            nc.gpsimd.affine_select(
                out=M01[:], in_=M01[:], compare_op=OP.is_ge, fill=0.0,
                base=0, pattern=[[1, 128]], channel_multiplier=-1,
            )

            # QT per head parity, zero-padded on the other head's rows
            QT0 = w.tile([128, PAIRS, T], BF16, tag="QT0")
            QT1 = w.tile([128, PAIRS, T], BF16, tag="QT1")
            nc.vector.memset(QT0[:], 0.0)
            nc.vector.memset(QT1[:], 0.0)
            KT = w.tile([128, PAIRS, T], BF16)
            # V with ones column: [t-part, kt, head, 65]
            VS = w.tile([128, NKT, H_CORE, 65], BF16)
            nc.vector.memset(VS[:, :, :, 64], 1.0)
            # O^T, heads pair-stacked: [d-part, pair, t]
            OT = w.tile([128, PAIRS, T], BF16, tag="OT")

            # ---- phase 1: QKV projections ----
            with tc.tile_pool(name="psA", bufs=2, space="PSUM") as psA:
                for p in range(PAIRS):
                    for ts in range(NT4):
                        sl = bass.ts(ts, 512)
                        pq = psA.tile([128, 512], F32, tag="pq")
                        pk = psA.tile([128, 512], F32, tag="pk")
                        for e in range(EC):
                            nc.tensor.matmul(
                                pq[:], WQ[:, e, bass.ts(p, 128)], XT[:, e, sl],
                                start=(e == 0), stop=(e == EC - 1))
                        for e in range(EC):
                            nc.tensor.matmul(
                                pk[:], WK[:, e, bass.ts(p, 128)], XT[:, e, sl],
                                start=(e == 0), stop=(e == EC - 1))
                        nc.scalar.activation(QT0[0:64, p, sl], pq[0:64, :],
                                             AF.Identity, bias=BQ[0:64, p:p + 1])
                        nc.scalar.activation(QT1[64:128, p, sl], pq[64:128, :],
                                             AF.Identity, bias=BQ[64:128, p:p + 1])
                        nc.scalar.activation(KT[:, p, sl], pk[:], AF.Identity,
                                             bias=BK[:, p:p + 1])
                for tt in range(NKT):
                    pv = psA.tile([128, 256], F32, tag="pv")
                    for e in range(EC):
                        nc.tensor.matmul(
                            pv[:], XT[:, e, bass.ts(tt, 128)], WV[:, e, :],
                            start=(e == 0), stop=(e == EC - 1))
                    nc.vector.tensor_tensor(
                        VS[:, tt, :, 0:64],
                        pv[:].rearrange("p (h d) -> p h d", h=H_CORE),
                        BVB[:].rearrange("p (h d) -> p h d", h=H_CORE),
                        OP.add)

            # ---- phase 2: attention (S^T layout) ----
            with (
                tc.tile_pool(name="psS", bufs=2, space="PSUM") as psS,
                tc.tile_pool(name="psO", bufs=1, space="PSUM") as psO,
            ):
                QTh = (QT0, QT1)
                for p in range(PAIRS):
                    for qs in range(NT4):
                        qsl = bass.ts(qs, 512)
                        ktmax = 4 * qs + 4
                        pO = [psO.tile([128, 512], F32, tag=f"pO{h}", name=f"pO{h}")
                              for h in range(2)]

                        def s_mm(kt):
                            d = kt - 4 * qs
                            lo = 128 * d if d > 0 else 0
                            res = []
                            for h in range(2):
                                ps = psS.tile([128, 512], F32, tag=f"pS{h}", name=f"pS{h}")
                                nc.tensor.matmul(
                                    ps[:, lo:512],
                                    KT[:, p, bass.ts(kt, 128)],
                                    QTh[h][:, p, bass.ds(qs * 512 + lo, 512 - lo)],
                                    start=True, stop=True)
                                res.append(ps)
                            return res

                        pS_cur = s_mm(0)
                        for kt in range(ktmax):
                            d = kt - 4 * qs
                            lo = 128 * d if d > 0 else 0
                            pS_nxt = s_mm(kt + 1) if kt + 1 < ktmax else None
                            for h in range(2):
                                head = 2 * p + h
                                PT = ptp.tile([128, 512], BF16, tag=f"PT{h}", name=f"PT{h}")
                                if lo > 0:
                                    nc.vector.memset(PT[:, 0:lo], 0.0)
                                nc.scalar.activation(
                                    PT[:, lo:512], pS_cur[h][:, lo:512], AF.Exp)
                                if d >= 0:
                                    nc.vector.tensor_tensor(
                                        PT[:, lo:lo + 128], PT[:, lo:lo + 128],
                                        M01[:], OP.mult)
                                nc.tensor.matmul(
                                    pO[h][0:65, :], VS[:, kt, head, :], PT[:],
                                    start=(kt == 0), stop=(kt == ktmax - 1))
                            pS_cur = pS_nxt

                        for h in range(2):
                            RL = misc.tile([1, 512], F32, tag="RL")
                            nc.vector.reciprocal(RL[:], pO[h][64:65, :])
                            RLB = misc.tile([64, 512], F32, tag="RLB")
                            nc.gpsimd.partition_broadcast(RLB[:], RL[:])
                            nc.vector.tensor_tensor(
                                OT[bass.ts(h, 64), p, qsl], pO[h][0:64, :],
                                RLB[:], OP.mult)

            # ---- phase 3: output projection (partial) ----
            with tc.tile_pool(name="psU", bufs=2, space="PSUM") as psU:
                for tt in range(NKT):
                    for ns in range(2):
                        pu = psU.tile([128, 512], F32, tag="pu")
                        for jc in range(2):
                            nc.tensor.matmul(
                                pu[:], OT[:, jc, bass.ts(tt, 128)],
                                WO[:, jc, bass.ts(ns, 512)],
                                start=(jc == 0), stop=(jc == 1))
                        ob = obp.tile([128, 512], F32, tag="ob")
                        nc.vector.tensor_tensor(
                            ob[:], pu[:], BOB[:, bass.ts(ns, 512)], OP.add)
                        nc.sync.dma_start(
                            out[bass.ts(tt, 128), bass.ts(ns, 512)], ob[:])

    nc.compile()
    return nc


def _in_maps(x, W_qkv, b_qkv, W_out, b_out):
    bf = ml_dtypes.bfloat16
    scale = np.float32(1.0 / np.sqrt(D))
    maps = []
    for c in range(8):
        b, hg = c // 4, c % 4
        qc = slice(hg * 256, hg * 256 + 256)
        m = {
            "xT": np.ascontiguousarray(x[b].T).astype(bf),
            "wq": (W_qkv[:, qc.start:qc.stop] * scale).astype(bf),
            "wk": W_qkv[:, E + qc.start:E + qc.stop].astype(bf),
            "wv": W_qkv[:, 2 * E + qc.start:2 * E + qc.stop].astype(bf),
            "wo": np.ascontiguousarray(W_out[qc, :]).astype(bf),
            "bq": (b_qkv[qc] * scale).astype(np.float32).reshape(2, 128).T.copy(),
            "bk": b_qkv[E + qc.start:E + qc.stop].astype(np.float32).reshape(2, 128).T.copy(),
            "bv": b_qkv[2 * E + qc.start:2 * E + qc.stop].astype(np.float32).reshape(1, 256).copy(),
            "bo": (b_out.astype(np.float32) if hg == 0
                   else np.zeros(E, np.float32)).reshape(1, E).copy(),
        }
        maps.append(m)
    return maps


def kernel(x, W_qkv, b_qkv, W_out, b_out):
    x = np.asarray(x, np.float32)
    W_qkv = np.asarray(W_qkv, np.float32)
    b_qkv = np.asarray(b_qkv, np.float32)
    W_out = np.asarray(W_out, np.float32)
    b_out = np.asarray(b_out, np.float32)
    if "nc" not in _cache:
        _cache["nc"] = _build()
    nc = _cache["nc"]
    maps = _in_maps(x, W_qkv, b_qkv, W_out, b_out)
    res = run_bass_kernel_spmd(nc, maps, list(range(8))).results
    out = np.empty((2, T, E), np.float32)
    for b in range(2):
        acc = res[b * 4]["out"].astype(np.float32)
        for hg in range(1, 4):
            acc = acc + res[b * 4 + hg]["out"]
        out[b] = acc
    return out


# revision 19
# speedup vs baseline: 25871.5456x; 25871.5456x over previous
# Causal self-attention on 8 NeuronCores (Trainium2, Bass/Tile).
#
# Sharding: core c -> batch b = c//4, head-group hg = c%4 (4 of 16 heads).
# Each core computes Q/K/V projections for its heads, causal attention, and
# a partial output projection (its heads' rows of W_out). Host sums the 4
# partials per batch (Megatron-style TP combine) and adds nothing else
# (b_out is folded in via the `bo` input on hg==0 cores).
#
# Device layouts (host-prepped, bf16 matmul operands):
#   xT  [1024, 2048]  x[b].T            -> rhs/lhsT chunks over e
#   wq/wk/wv [1024, 256]  W_qkv col-slices (wq pre-scaled by 1/sqrt(D))
#   wo  [256, 1024]   W_out row-slice
# Attention runs in S^T = K Q^T layout ([k, q]); the PV matmul uses
# lhsT = [V | 1] so PSUM row 64 accumulates the softmax denominator.

import os
import numpy as np
import ml_dtypes

import concourse.bass as bass
import concourse.mybir as mybir
import concourse.tile as tile
from concourse import bacc
from concourse.bass_utils import run_bass_kernel_spmd

F32 = mybir.dt.float32
BF16 = mybir.dt.bfloat16
AF = mybir.ActivationFunctionType
OP = mybir.AluOpType

T = 2048
E = 1024
D = 64
NH = 16
H_CORE = 4          # heads per core
PAIRS = 2           # head pairs per core
EC = E // 128       # 8 e-chunks
NT4 = T // 512      # 4 t-tiles of 512
NKT = T // 128      # 16 k-tiles of 128

_cache = {}


def _build(reps=1):
    nc = bacc.Bacc(None, target_bir_lowering=False)
    xT = nc.declare_dram_parameter("xT", [E, T], BF16, isOutput=False)
    wq = nc.declare_dram_parameter("wq", [E, 256], BF16, isOutput=False)
    wk = nc.declare_dram_parameter("wk", [E, 256], BF16, isOutput=False)
    wv = nc.declare_dram_parameter("wv", [E, 256], BF16, isOutput=False)
    wo = nc.declare_dram_parameter("wo", [256, E], BF16, isOutput=False)
    bq = nc.declare_dram_parameter("bq", [128, 2], F32, isOutput=False)
    bk = nc.declare_dram_parameter("bk", [128, 2], F32, isOutput=False)
    bv = nc.declare_dram_parameter("bv", [1, 256], F32, isOutput=False)
    bo = nc.declare_dram_parameter("bo", [1, E], F32, isOutput=False)
    out = nc.declare_dram_parameter("out", [T, E], F32, isOutput=True)

    xT_r = xT.rearrange("(c p) t -> p c t", p=128)
    wq_r = wq.rearrange("(c p) m -> p c m", p=128)
    wk_r = wk.rearrange("(c p) m -> p c m", p=128)
    wv_r = wv.rearrange("(c p) m -> p c m", p=128)
    wo_r = wo.rearrange("(c p) n -> p c n", p=128)

    import contextlib

    with tile.TileContext(nc) as tc:
        with (
            tc.tile_pool(name="w", bufs=1) as w,
            tc.tile_pool(name="pt", bufs=6) as ptp,
            tc.tile_pool(name="misc", bufs=4) as misc,
            tc.tile_pool(name="ob", bufs=3) as obp,
            tc.For_i(0, reps, 1) if reps > 1 else contextlib.nullcontext(),
        ):
            # ---- static tiles ----
            XT = w.tile([128, EC, T], BF16)
            WQ = w.tile([128, EC, 256], BF16)
            WK = w.tile([128, EC, 256], BF16)
            WV = w.tile([128, EC, 256], BF16)
            WO = w.tile([128, 2, E], BF16)
            BQ = w.tile([128, 2], F32)
            BK = w.tile([128, 2], F32)
            BV1 = w.tile([1, 256], F32)
            BO1 = w.tile([1, E], F32)
            # critical-path-first loads, split across both HWDGE engines
            # (ACT is idle during phase 1)
            for c in range(EC):
                nc.scalar.dma_start(WQ[:, c, :], wq_r[:, c, :])
                nc.sync.dma_start(XT[:, c, bass.ts(0, 512)],
                                  xT_r[:, c, bass.ts(0, 512)])
            for c in range(EC):
                nc.scalar.dma_start(WK[:, c, :], wk_r[:, c, :])
            nc.scalar.dma_start(BQ[:], bq[:])
            nc.scalar.dma_start(BK[:], bk[:])
            nc.scalar.dma_start(BV1[:], bv[:])
            nc.scalar.dma_start(BO1[:], bo[:])
            nc.scalar.dma_start(WV[:], wv_r[:])
            for ts in range(1, NT4):
                for c in range(EC):
                    nc.sync.dma_start(XT[:, c, bass.ts(ts, 512)],
                                      xT_r[:, c, bass.ts(ts, 512)])
            nc.scalar.dma_start(WO[:], wo_r[:])
            BVB = w.tile([128, 256], F32)
            nc.gpsimd.partition_broadcast(BVB[:], BV1[:])
            BOB = w.tile([128, E], F32)
            nc.gpsimd.partition_broadcast(BOB[:], BO1[:])

            # triangular keep-mask [k, j]: keep j >= k
            M01 = w.tile([128, 128], BF16)
            nc.vector.memset(M01[:], 1.0)
            nc.gpsimd.affine_select(
                out=M01[:], in_=M01[:], compare_op=OP.is_ge, fill=0.0,
                base=0, pattern=[[1, 128]], channel_multiplier=-1,
            )

            QT = w.tile([128, PAIRS, T], BF16, tag="QT")
            KT = w.tile([128, PAIRS, T], BF16)
            # V with ones column: [t-part, kt, head, 65]
            VS = w.tile([128, NKT, H_CORE, 65], BF16)
            nc.gpsimd.memset(VS[:, :, :, 64], 1.0)
            # O^T, heads pair-stacked: [d-part, pair, t]
            OT = w.tile([128, PAIRS, T], BF16, tag="OT")

            # ---- phase 1: QKV projections (t-sliced so ts=0 starts early) ----
            with tc.tile_pool(name="psA", bufs=3, space="PSUM") as psA:
                for ts in range(NT4):
                    sl = bass.ts(ts, 512)
                    for p in range(PAIRS):
                        pq = psA.tile([128, 512], F32, tag="qk", name="pq")
                        for e in range(EC):
                            nc.tensor.matmul(
                                pq[:], WQ[:, e, bass.ts(p, 128)], XT[:, e, sl],
                                start=(e == 0), stop=(e == EC - 1))
                        pk = psA.tile([128, 512], F32, tag="qk", name="pk")
                        for e in range(EC):
                            nc.tensor.matmul(
                                pk[:], WK[:, e, bass.ts(p, 128)], XT[:, e, sl],
                                start=(e == 0), stop=(e == EC - 1))
                        nc.vector.tensor_scalar_add(QT[:, p, sl], pq[:],
                                                    BQ[:, p:p + 1])
                        nc.vector.tensor_scalar_add(KT[:, p, sl], pk[:],
                                                    BK[:, p:p + 1])
                    for tt in range(4 * ts, 4 * ts + 4):
                        pv = psA.tile([128, 256], F32, tag="pv")
                        for e in range(EC):
                            nc.tensor.matmul(
                                pv[:], XT[:, e, bass.ts(tt, 128)], WV[:, e, :],
                                start=(e == 0), stop=(e == EC - 1))
                        nc.vector.tensor_tensor(
                            VS[:, tt, :, 0:64],
                            pv[:].rearrange("p (h d) -> p h d", h=H_CORE),
                            BVB[:].rearrange("p (h d) -> p h d", h=H_CORE),
                            OP.add)

            # ---- phase 2+3 fused: attention, then out-proj for that q-block ----
            with (
                tc.tile_pool(name="psS", bufs=2, space="PSUM") as psS,
                tc.tile_pool(name="psO", bufs=2, space="PSUM") as psO,
            ):
                for qs in range(NT4):
                    qsl = bass.ts(qs, 512)
                    ktmax = 4 * qs + 4
                    for p in range(PAIRS):
                        pO = [psO.tile([128, 512], F32, tag=f"pO{h}", name=f"pO{h}")
                              for h in range(2)]

                        # pass A: scores (row-packed K=64, heads concurrent) + exp
                        PTs = {}
                        for kt in range(ktmax):
                            d = kt - 4 * qs
                            lo = 128 * d if d > 0 else 0
                            for h in range(2):
                                ps = psS.tile([128, 512], F32, tag=f"pS{h}",
                                              name=f"pS{h}")
                                nc.tensor.matmul(
                                    ps[:, lo:512],
                                    KT[bass.ts(h, 64), p, bass.ts(kt, 128)],
                                    QT[bass.ts(h, 64), p,
                                       bass.ds(qs * 512 + lo, 512 - lo)],
                                    start=True, stop=True)
                                PT = ptp.tile([128, 512], BF16, tag=f"PT{h}",
                                              name=f"PT{h}", bufs=16)
                                if lo > 0:
                                    nc.vector.memset(PT[:, 0:lo], 0.0)
                                nc.scalar.activation(
                                    PT[:, lo:512], ps[:, lo:512], AF.Exp)
                                if d >= 0:
                                    nc.vector.tensor_tensor(
                                        PT[:, lo:lo + 128], PT[:, lo:lo + 128],
                                        M01[:], OP.mult)
                                PTs[(kt, h)] = PT
                        # pass B: PV accumulation (K=128 mode)
                        for kt in range(ktmax):
                            for h in range(2):
                                nc.tensor.matmul(
                                    pO[h][0:65, :], VS[:, kt, 2 * p + h, :],
                                    PTs[(kt, h)][:],
                                    start=(kt == 0), stop=(kt == ktmax - 1))

                        for h in range(2):
                            RL = misc.tile([1, 512], F32, tag="RL")
                            nc.vector.reciprocal(RL[:], pO[h][64:65, :])
                            RLB = misc.tile([64, 512], F32, tag="RLB")
                            nc.gpsimd.partition_broadcast(RLB[:], RL[:])
                            nc.vector.tensor_tensor(
                                OT[bass.ts(h, 64), p, qsl], pO[h][0:64, :],
                                RLB[:], OP.mult)

                    # out-proj for the q-block just finished
                    for tt in range(4 * qs, 4 * qs + 4):
                        for ns in range(2):
                            pu = psS.tile([128, 512], F32, tag="pS0", name="pu")
                            for jc in range(2):
                                nc.tensor.matmul(
                                    pu[:], OT[:, jc, bass.ts(tt, 128)],
                                    WO[:, jc, bass.ts(ns, 512)],
                                    start=(jc == 0), stop=(jc == 1))
                            ob = obp.tile([128, 512], F32, tag="ob")
                            nc.vector.tensor_tensor(
                                ob[:], pu[:], BOB[:, bass.ts(ns, 512)], OP.add)
                            nc.sync.dma_start(
                                out[bass.ts(tt, 128), bass.ts(ns, 512)], ob[:])

    nc.compile()
    return nc


def _in_maps(x, W_qkv, b_qkv, W_out, b_out):
    bf = ml_dtypes.bfloat16
    scale = np.float32(1.0 / np.sqrt(D))
    maps = []
    for c in range(8):
        b, hg = c // 4, c % 4
        qc = slice(hg * 256, hg * 256 + 256)
        m = {
            "xT": np.ascontiguousarray(x[b].T).astype(bf),
            "wq": (W_qkv[:, qc.start:qc.stop] * scale).astype(bf),
            "wk": W_qkv[:, E + qc.start:E + qc.stop].astype(bf),
            "wv": W_qkv[:, 2 * E + qc.start:2 * E + qc.stop].astype(bf),
            "wo": np.ascontiguousarray(W_out[qc, :]).astype(bf),
            "bq": (b_qkv[qc] * scale).astype(np.float32).reshape(2, 128).T.copy(),
            "bk": b_qkv[E + qc.start:E + qc.stop].astype(np.float32).reshape(2, 128).T.copy(),
            "bv": b_qkv[2 * E + qc.start:2 * E + qc.stop].astype(np.float32).reshape(1, 256).copy(),
            "bo": (b_out.astype(np.float32) if hg == 0
                   else np.zeros(E, np.float32)).reshape(1, E).copy(),
        }
        maps.append(m)
    return maps


def kernel(x, W_qkv, b_qkv, W_out, b_out):
    x = np.asarray(x, np.float32)
    W_qkv = np.asarray(W_qkv, np.float32)
    b_qkv = np.asarray(b_qkv, np.float32)
    W_out = np.asarray(W_out, np.float32)
    b_out = np.asarray(b_out, np.float32)
    if "nc" not in _cache:
        _cache["nc"] = _build()
    nc = _cache["nc"]
    maps = _in_maps(x, W_qkv, b_qkv, W_out, b_out)
    res = run_bass_kernel_spmd(nc, maps, list(range(8))).results
    out = np.empty((2, T, E), np.float32)
    for b in range(2):
        acc = res[b * 4]["out"].astype(np.float32)
        for hg in range(1, 4):
            acc = acc + res[b * 4 + hg]["out"]
        out[b] = acc
    return out
